# revision 1
# baseline (speedup 1.0000x reference)
"""Trainium2 Bass kernel for nn_Attention_15539191677265.

Single-head-dim attention block:
    qkv = w_qkv @ x ; per-head scaled dot-product attention over w=2048;
    out = w_out @ attn_out + b_out

Sharding: pure data-parallel over batch (b=8 -> 8 NeuronCores, one batch
element per core). Weights are replicated. No collectives.

Per-core algorithm (transposed-softmax scheme, all matmuls bf16):
  1. q,k = wqkvT.T @ x           ([c,o] stationary; q pre-scaled on host)
  2. vT  = x.T @ wvT             (v produced directly transposed [j, d])
  3. per head: sim^T[j,i] strips -> exp on ScalarE (no max subtraction:
     scores are ~N(0,1), exp cannot overflow in fp32/bf16 range)
  4. AV: out^T[d,i] = vT.T @ exp_strip, with a ones-column appended to vT
     so row 64 of the psum accumulates the softmax normalizer for free
  5. normalize: reciprocal(norm row) -> partition_broadcast -> multiply
  6. proj: out = woutT.T @ attn_out (per-head K=64 chunks) + bias
"""

import sys

if "/opt/trn_rl_repo" not in sys.path:
    sys.path.insert(0, "/opt/trn_rl_repo")

import numpy as np
import ml_dtypes

import concourse.bass as bass
import concourse.mybir as mybir
import concourse.tile as tile
from concourse import bacc
from concourse.bass_utils import run_bass_kernel_spmd

BF16 = mybir.dt.bfloat16
F32 = mybir.dt.float32
EXP = mybir.ActivationFunctionType.Exp

B, DIM, W = 8, 256, 2048
HEADS, DH = 8, 64
HID = HEADS * DH  # 512
SCALE = DH ** (-0.5)
N_CORES = 8

NJT = W // 128  # 16 j-tiles per head
NCT = DIM // 128  # 2 contraction chunks over channels


def build_kernel():
    nc = bacc.Bacc(None, target_bir_lowering=False)

    x_d = nc.dram_tensor("x", [DIM, W], BF16, kind="ExternalInput")
    wqkvT_d = nc.dram_tensor("wqkvT", [DIM, 3 * HID], BF16, kind="ExternalInput")
    woutT_d = nc.dram_tensor("woutT", [128, 4, DIM], BF16, kind="ExternalInput")
    bias_d = nc.dram_tensor("bias", [128, DIM // 128], F32, kind="ExternalInput")
    out_d = nc.dram_tensor("out", [DIM, W], F32, kind="ExternalOutput")

    with tile.TileContext(nc) as tc:
        with tc.tile_pool(name="pers", bufs=1) as pers:
            x_sb = pers.tile([128, NCT, W], BF16)
            wq_sb = pers.tile([128, NCT, 3 * HID], BF16)
            wo_sb = pers.tile([128, 4, DIM], BF16)
            bias_sb = pers.tile([128, DIM // 128], F32)
            q_sb = pers.tile([128, 4, W], BF16)
            k_sb = pers.tile([128, 4, W], BF16)
            vt_sb = pers.tile([128, NJT, HEADS, 128], BF16)
            attout_sb = [
                pers.tile([128, W], BF16, name=f"attout_{kc}", tag=f"attout{kc}")
                for kc in range(4)
            ]
            out_sb = pers.tile([128, NCT, W], F32)

            xr = x_d[:].rearrange("(ct p) w -> p ct w", p=128)
            for ct in range(NCT):
                for wh in range(4):
                    nc.sync.dma_start(
                        out=x_sb[:, ct, wh * 512 : (wh + 1) * 512],
                        in_=xr[:, ct, wh * 512 : (wh + 1) * 512],
                    )
            wqr = wqkvT_d[:].rearrange("(ct p) o -> p ct o", p=128)
            for ct in range(NCT):
                for sec in range(3):
                    nc.sync.dma_start(
                        out=wq_sb[:, ct, sec * HID : (sec + 1) * HID],
                        in_=wqr[:, ct, sec * HID : (sec + 1) * HID],
                    )
            nc.sync.dma_start(out=wo_sb[:], in_=woutT_d[:])
            nc.sync.dma_start(out=bias_sb[:], in_=bias_d[:])

            # cols 64..127: ones column then zero padding (FWL needs 128)
            nc.vector.memset(vt_sb[:, :, :, DH:128], 0.0)
            nc.vector.memset(vt_sb[:, :, :, DH : DH + 1], 1.0)
            # warm the ACT exp table set while qkv matmuls run
            warm = pers.tile([1, 1], F32)
            nc.vector.memset(warm[:], 0.0)
            nc.scalar.activation(out=warm[:], in_=warm[:], func=EXP)

            # ---- phase 1: q, k projections -> [128, 4, W] bf16 each ----
            # [128,1024] psum pieces with 4-slot rotation: evacuation copies
            # (alternating DVE/ACT) never stall the o-tile matmul stream
            with tc.tile_pool(name="qkv_ps", bufs=8, space="PSUM") as qkv_ps:
                for ot in range(8):  # o-tiles 0..3 = q, 4..7 = k
                    dst = q_sb if ot < 4 else k_sb
                    for ph in range(4):
                        po = ph * 512
                        ps = qkv_ps.tile(
                            [128, 512], F32, name=f"qk_{ot}_{ph}", tag="qk"
                        )
                        for ct in range(NCT):
                            nc.tensor.matmul(
                                ps[:],
                                lhsT=wq_sb[:, ct, ot * 128 : (ot + 1) * 128],
                                rhs=x_sb[:, ct, po : po + 512],
                                start=(ct == 0),
                                stop=(ct == NCT - 1),
                            )
                        if (4 * ot + ph) % 2 == 0:
                            nc.vector.tensor_copy(
                                out=dst[:, ot % 4, po : po + 512], in_=ps[:]
                            )
                        else:
                            nc.scalar.copy(
                                out=dst[:, ot % 4, po : po + 512], in_=ps[:]
                            )

            # ---- phase 2: vT[j, hd] = x.T @ wvT  (+ones col kept) ----
            with tc.tile_pool(name="vt_ps", bufs=8, space="PSUM") as vt_ps:
                for jt in range(NJT):
                    ps = vt_ps.tile([128, HID], F32)
                    for ct in range(NCT):
                        nc.tensor.matmul(
                            ps[:],
                            lhsT=x_sb[:, ct, jt * 128 : (jt + 1) * 128],
                            rhs=wq_sb[:, ct, 2 * HID : 3 * HID],
                            start=(ct == 0),
                            stop=(ct == NCT - 1),
                        )
                    if jt % 2 == 0:
                        nc.vector.tensor_copy(
                            out=vt_sb[:, jt, :, 0:DH],
                            in_=ps[:].rearrange("p (h d) -> p h d", h=HEADS),
                        )
                    else:
                        nc.scalar.copy(
                            out=vt_sb[:, jt, :, 0:DH],
                            in_=ps[:].rearrange("p (h d) -> p h d", h=HEADS),
                        )

            # ---- phase 3: attention per head ----
            with (
                tc.tile_pool(name="strip_ps", bufs=1, space="PSUM") as strip_ps,
                tc.tile_pool(name="av_ps", bufs=2, space="PSUM") as av_ps,
                tc.tile_pool(name="exp_sb", bufs=10) as exp_pool,
                tc.tile_pool(name="norm_sb", bufs=4) as norm_pool,
            ):
                for h in range(HEADS):
                    kt, koff = h // 2, (h % 2) * 64
                    avs = [
                        av_ps.tile([128, 1024], F32, name=f"av_{h}_{ih}", tag="av")
                        for ih in range(2)
                    ]
                    for jt in range(NJT):
                        # 4 sim matmuls sharing one k-tile LDWEIGHTS, then
                        # 4 AV matmuls sharing one vT LDWEIGHTS
                        strips = []
                        for ih in range(2):
                            io = ih * 1024
                            strip = strip_ps.tile(
                                [128, 1024], F32, name=f"st_{ih}", tag=f"st{ih}"
                            )
                            strips.append(strip)
                            for ns in range(2):
                                nc.tensor.matmul(
                                    strip[:, ns * 512 : (ns + 1) * 512],
                                    lhsT=k_sb[
                                        koff : koff + DH, kt, jt * 128 : (jt + 1) * 128
                                    ],
                                    rhs=q_sb[
                                        koff : koff + DH,
                                        kt,
                                        io + ns * 512 : io + (ns + 1) * 512,
                                    ],
                                    start=True,
                                    stop=True,
                                )
                        ess = []
                        for ih in range(2):
                            es = exp_pool.tile(
                                [128, 1024], BF16, name=f"es_{ih}", tag="es"
                            )
                            ess.append(es)
                            nc.scalar.activation(
                                out=es[:], in_=strips[ih][:], func=EXP
                            )
                        for ih in range(2):
                            for ns in range(2):
                                nc.tensor.matmul(
                                    avs[ih][:, ns * 512 : (ns + 1) * 512],
                                    lhsT=vt_sb[:, jt, h, :],
                                    rhs=ess[ih][:, ns * 512 : (ns + 1) * 512],
                                    start=(jt == 0),
                                    stop=(jt == NJT - 1),
                                )
                    for ih in range(2):
                        io = ih * 1024
                        av = avs[ih]
                        # evacuate psum right away so the av slot frees for the
                        # next head; the norm chain then runs off-critical-path
                        avc = norm_pool.tile([DH + 1, 1024], F32, tag="avc")
                        nc.vector.tensor_copy(out=avc[:], in_=av[0 : DH + 1, :])
                        rec0 = norm_pool.tile([1, 1024], F32, tag="rec0")
                        bcn = norm_pool.tile([DH, 1024], F32, tag="bcn")
                        bc = norm_pool.tile([DH, 1024], F32, tag="bc")
                        # partition_broadcast + custom-DVE ops only work from
                        # partition 0 -> DMA the raw norm row there first
                        nc.sync.dma_start(out=rec0[:], in_=avc[DH : DH + 1, :])
                        nc.gpsimd.partition_broadcast(
                            bcn[:], rec0[0:1, :], channels=DH
                        )
                        nc.vector.reciprocal_approx_fast(out=bc[:], in_=bcn[:])
                        if h % 2 == 0:
                            nc.vector.tensor_mul(
                                out=attout_sb[h // 2][0:DH, io : io + 1024],
                                in0=avc[0:DH, :],
                                in1=bc[:],
                            )
                        else:
                            # odd heads land on partitions 64..127: DVE cannot
                            # shift partitions, so write via a bounce + DMA
                            atmp = norm_pool.tile([DH, 1024], BF16, tag="atmp")
                            nc.vector.tensor_mul(
                                out=atmp[:], in0=avc[0:DH, :], in1=bc[:]
                            )
                            nc.sync.dma_start(
                                out=attout_sb[h // 2][DH:128, io : io + 1024],
                                in_=atmp[:],
                            )

            # ---- phase 4: output projection + bias (K=128 head pairs) ----
            outr = out_d[:].rearrange("(ct p) w -> p ct w", p=128)
            with tc.tile_pool(name="proj_ps", bufs=8, space="PSUM") as proj_ps:
                for ot in range(NCT):
                    for wh in range(4):
                        wo = wh * 512
                        ps = proj_ps.tile(
                            [128, 512], F32, name=f"pj_{ot}_{wh}", tag="pj"
                        )
                        for kc in range(4):
                            nc.tensor.matmul(
                                ps[:],
                                lhsT=wo_sb[:, kc, ot * 128 : (ot + 1) * 128],
                                rhs=attout_sb[kc][:, wo : wo + 512],
                                start=(kc == 0),
                                stop=(kc == 3),
                            )
                        nc.vector.tensor_scalar_add(
                            out=out_sb[:, ot, wo : wo + 512],
                            in0=ps[:],
                            scalar1=bias_sb[:, ot : ot + 1],
                        )
                        nc.sync.dma_start(
                            out=outr[:, ot, wo : wo + 512],
                            in_=out_sb[:, ot, wo : wo + 512],
                        )

    nc.compile()
    return nc



_NC_CACHE = None


def _get_nc():
    global _NC_CACHE
    if _NC_CACHE is None:
        _NC_CACHE = build_kernel()
    return _NC_CACHE


def make_in_maps(x, w_qkv, w_out, b_out):
    bf16 = ml_dtypes.bfloat16
    wq = np.array(w_qkv, dtype=np.float32, copy=True)
    wq[:HID] *= SCALE  # fold attention scale into the q projection
    wqkvT = np.ascontiguousarray(wq.T).astype(bf16)  # [256, 1536]
    woutT = np.ascontiguousarray(
        w_out.T.reshape(4, 128, DIM).transpose(1, 0, 2)
    ).astype(bf16)  # [128, 4, 256]
    bias = np.ascontiguousarray(
        b_out.astype(np.float32).reshape(DIM // 128, 128).T
    )  # [128, 2]
    in_maps = []
    for i in range(N_CORES):
        in_maps.append(
            {
                "x": x[i].astype(bf16),
                "wqkvT": wqkvT,
                "woutT": woutT,
                "bias": bias,
            }
        )
    return in_maps


def kernel(x, w_qkv, w_out, b_out, _trace=False):
    nc = _get_nc()
    in_maps = make_in_maps(x, w_qkv, w_out, b_out)
    res = run_bass_kernel_spmd(
        nc,
        in_maps,
        core_ids=list(range(N_CORES)),
        trace=_trace,
        trace_cores=list(range(N_CORES)) if _trace else None,
    )
    out = np.stack([res.results[i]["out"] for i in range(N_CORES)], axis=0)
    if _trace:
        kernel.last_exec_time_ns = res.exec_time_ns
        kernel.last_results = res
    return out



# revision 4
# speedup vs baseline: 1.3413x; 1.3413x over previous
"""Trainium2 Bass kernel for nn_Attention_15539191677265.

Single-head-dim attention block:
    qkv = w_qkv @ x ; per-head scaled dot-product attention over w=2048;
    out = w_out @ attn_out + b_out

Sharding: pure data-parallel over batch (b=8 -> 8 NeuronCores, one batch
element per core). Weights are replicated. No collectives.

Per-core algorithm (transposed-softmax, bf16 matmuls, ACT-bound pipeline):
  1. q,k projections emit PARTITION-DUPLICATED copies (d-dims at
     partitions 0-63 and 64-127) via duplicated stationary weights, so the
     sim matmuls can run as two concurrent PE quadrant streams.
  2. vT = x.T @ wvT with a ones column appended (row 64 of the AV psum
     accumulates the softmax normalizer for free).
  3. per (head, i-half 1024, j-tile 128): sim^T strip via two K=64
     matmuls on PE quadrants (0,0) and (64,64) -> [128 j, 1024 i] psum;
     exp on ScalarE as ONE [128,1024] activation (2-bank psum span);
     AV accumulates [65, 1024] over the 16 j-tiles.
  4. normalize via reciprocal of the ones-row, broadcast multiply (DVE /
     GpSimd, off the critical path), bounce odd heads via DMA.
  5. proj: out = woutT.T @ attn_out (K=128 head-pair chunks) + bias.

No max subtraction before exp: scores are ~N(0,1) so exp cannot
overflow in bf16/fp32.
"""

import sys

if "/opt/trn_rl_repo" not in sys.path:
    sys.path.insert(0, "/opt/trn_rl_repo")

import numpy as np
import ml_dtypes

import concourse.bass as bass
import concourse.mybir as mybir
import concourse.tile as tile
from concourse import bacc
from concourse.bass_utils import run_bass_kernel_spmd

BF16 = mybir.dt.bfloat16
F32 = mybir.dt.float32
EXP = mybir.ActivationFunctionType.Exp

B, DIM, W = 8, 256, 2048
HEADS, DH = 8, 64
HID = HEADS * DH  # 512
SCALE = DH ** (-0.5)
N_CORES = 8

NJT = W // 128  # 16 j-tiles per head
NCT = DIM // 128  # 2 contraction chunks over channels
IH = 1024  # i-half width
NIH = W // IH  # 2 i-halves


def build_kernel():
    nc = bacc.Bacc(None, target_bir_lowering=False)

    x_d = nc.dram_tensor("x", [DIM, W], BF16, kind="ExternalInput")
    # host layout: [256, 2560] = [qdup 1024 | kdup 1024 | wv 512]
    wqkv_d = nc.dram_tensor("wqkv", [DIM, 2560], BF16, kind="ExternalInput")
    woutT_d = nc.dram_tensor("woutT", [128, 4, DIM], BF16, kind="ExternalInput")
    bias_d = nc.dram_tensor("bias", [128, DIM // 128], F32, kind="ExternalInput")
    out_d = nc.dram_tensor("out", [DIM, W], F32, kind="ExternalOutput")

    with tile.TileContext(nc) as tc:
        with tc.tile_pool(name="pers", bufs=1) as pers:
            x_sb = pers.tile([128, NCT, W], BF16)
            wq_sb = pers.tile([128, NCT, 2560], BF16)
            wo_sb = pers.tile([128, 4, DIM], BF16)
            bias_sb = pers.tile([128, DIM // 128], F32)
            # duplicated-partition layouts: [128 = d 0..63 twice, head, w]
            q_sb = pers.tile([128, HEADS, W], BF16)
            k_sb = pers.tile([128, HEADS, W], BF16)
            vt_sb = pers.tile([128, NJT, HEADS, 128], BF16)
            attout_sb = [
                pers.tile([128, W], BF16, name=f"attout_{kc}", tag=f"attout{kc}")
                for kc in range(4)
            ]
            out_sb = pers.tile([128, NCT, W], F32)

            xr = x_d[:].rearrange("(ct p) w -> p ct w", p=128)
            for ct in range(NCT):
                for wh in range(4):
                    nc.sync.dma_start(
                        out=x_sb[:, ct, wh * 512 : (wh + 1) * 512],
                        in_=xr[:, ct, wh * 512 : (wh + 1) * 512],
                    )
            wqr = wqkv_d[:].rearrange("(ct p) o -> p ct o", p=128)
            for ct in range(NCT):
                for sec in range(5):
                    nc.sync.dma_start(
                        out=wq_sb[:, ct, sec * 512 : (sec + 1) * 512],
                        in_=wqr[:, ct, sec * 512 : (sec + 1) * 512],
                    )
            nc.sync.dma_start(out=wo_sb[:], in_=woutT_d[:])
            nc.sync.dma_start(out=bias_sb[:], in_=bias_d[:])

            # vT cols 64..127: ones column then zero padding
            nc.vector.memset(vt_sb[:, :, :, DH:128], 0.0)
            nc.vector.memset(vt_sb[:, :, :, DH : DH + 1], 1.0)
            # warm the ACT exp table while qkv matmuls run
            warm = pers.tile([1, 1], F32)
            nc.vector.memset(warm[:], 0.0)
            nc.scalar.activation(out=warm[:], in_=warm[:], func=EXP)

            # ---- phase 1: q, k projections (partition-duplicated) ----
            with tc.tile_pool(name="qkv_ps", bufs=4, space="PSUM") as qkv_ps:
                for h in range(HEADS):
                    for dst, base in ((q_sb, 0), (k_sb, 1024)):
                        for ph in range(4):
                            po = ph * 512
                            ps = qkv_ps.tile([128, 512], F32, name=f"qk_{h}_{base}_{ph}", tag="qk")
                            for ct in range(NCT):
                                nc.tensor.matmul(
                                    ps[:],
                                    lhsT=wq_sb[:, ct, base + h * 128 : base + (h + 1) * 128],
                                    rhs=x_sb[:, ct, po : po + 512],
                                    start=(ct == 0),
                                    stop=(ct == NCT - 1),
                                )
                            if ph % 2 == 0:
                                nc.vector.tensor_copy(
                                    out=dst[:, h, po : po + 512], in_=ps[:]
                                )
                            else:
                                nc.scalar.copy(
                                    out=dst[:, h, po : po + 512], in_=ps[:]
                                )

                # ---- phase 2: vT[j, hd] = x.T @ wvT (+ones col kept) ----
                for jt in range(NJT):
                    ps = qkv_ps.tile([128, HID], F32, name=f"vt_{jt}", tag="vt")
                    for ct in range(NCT):
                        nc.tensor.matmul(
                            ps[:],
                            lhsT=x_sb[:, ct, jt * 128 : (jt + 1) * 128],
                            rhs=wq_sb[:, ct, 2048:2560],
                            start=(ct == 0),
                            stop=(ct == NCT - 1),
                        )
                    if jt % 2 == 0:
                        nc.vector.tensor_copy(
                            out=vt_sb[:, jt, :, 0:DH],
                            in_=ps[:].rearrange("p (h d) -> p h d", h=HEADS),
                        )
                    else:
                        nc.scalar.copy(
                            out=vt_sb[:, jt, :, 0:DH],
                            in_=ps[:].rearrange("p (h d) -> p h d", h=HEADS),
                        )

            # ---- phase 3: attention ----
            with (
                tc.tile_pool(name="strip_ps", bufs=2, space="PSUM") as strip_ps,
                tc.tile_pool(name="av_ps", bufs=2, space="PSUM") as av_ps,
                tc.tile_pool(name="exp_sb", bufs=3) as exp_pool,
                tc.tile_pool(name="norm_sb", bufs=2) as norm_pool,
            ):
                for h in range(HEADS):
                    for ih in range(NIH):
                        io = ih * IH
                        av = av_ps.tile([128, IH], F32, name=f"av_{h}_{ih}", tag="av")
                        for jt in range(NJT):
                            strip = strip_ps.tile(
                                [128, IH], F32, name=f"st_{h}_{ih}_{jt}", tag="st"
                            )
                            for c in range(IH // 512):
                                co = c * 512
                                # two concurrent quadrant streams
                                nc.tensor.matmul(
                                    strip[0:64, co : co + 512],
                                    lhsT=k_sb[0:64, h, jt * 128 : jt * 128 + 64],
                                    rhs=q_sb[0:64, h, io + co : io + co + 512],
                                    start=True,
                                    stop=True,
                                    tile_position=(0, 0),
                                )
                                nc.tensor.matmul(
                                    strip[64:128, co : co + 512],
                                    lhsT=k_sb[64:128, h, jt * 128 + 64 : (jt + 1) * 128],
                                    rhs=q_sb[64:128, h, io + co : io + co + 512],
                                    start=True,
                                    stop=True,
                                    tile_position=(64, 64),
                                )
                            es = exp_pool.tile([128, IH], BF16, name=f"es_{jt}", tag="es")
                            nc.scalar.activation(out=es[:], in_=strip[:], func=EXP)
                            for c in range(IH // 512):
                                co = c * 512
                                nc.tensor.matmul(
                                    av[0 : DH + 1, co : co + 512],
                                    lhsT=vt_sb[:, jt, h, 0 : DH + 1],
                                    rhs=es[:, co : co + 512],
                                    start=(jt == 0),
                                    stop=(jt == NJT - 1),
                                )
                        # normalize off the critical path
                        avc = norm_pool.tile([DH + 1, IH], F32, tag="avc")
                        nc.vector.tensor_copy(out=avc[:], in_=av[0 : DH + 1, :])
                        rec0 = norm_pool.tile([1, IH], F32, tag="rec0")
                        bcn = norm_pool.tile([DH, IH], F32, tag="bcn")
                        bc = norm_pool.tile([DH, IH], F32, tag="bc")
                        nc.sync.dma_start(out=rec0[:], in_=avc[DH : DH + 1, :])
                        nc.gpsimd.partition_broadcast(bcn[:], rec0[0:1, :], channels=DH)
                        nc.vector.reciprocal_approx_fast(out=bc[:], in_=bcn[:])
                        if h % 2 == 0:
                            nc.vector.tensor_mul(
                                out=attout_sb[h // 2][0:DH, io : io + IH],
                                in0=avc[0:DH, :],
                                in1=bc[:],
                            )
                        else:
                            # odd heads land on partitions 64..127: DVE cannot
                            # shift partitions, so bounce + DMA
                            atmp = norm_pool.tile([DH, IH], BF16, tag="atmp")
                            nc.vector.tensor_mul(out=atmp[:], in0=avc[0:DH, :], in1=bc[:])
                            nc.sync.dma_start(
                                out=attout_sb[h // 2][DH:128, io : io + IH],
                                in_=atmp[:],
                            )

            # ---- phase 4: output projection + bias ----
            outr = out_d[:].rearrange("(ct p) w -> p ct w", p=128)
            with tc.tile_pool(name="proj_ps", bufs=8, space="PSUM") as proj_ps:
                for ot in range(NCT):
                    for wh in range(4):
                        wo = wh * 512
                        ps = proj_ps.tile([128, 512], F32, name=f"pj_{ot}_{wh}", tag="pj")
                        for kc in range(4):
                            nc.tensor.matmul(
                                ps[:],
                                lhsT=wo_sb[:, kc, ot * 128 : (ot + 1) * 128],
                                rhs=attout_sb[kc][:, wo : wo + 512],
                                start=(kc == 0),
                                stop=(kc == 3),
                            )
                        nc.vector.tensor_scalar_add(
                            out=out_sb[:, ot, wo : wo + 512],
                            in0=ps[:],
                            scalar1=bias_sb[:, ot : ot + 1],
                        )
                        nc.sync.dma_start(
                            out=outr[:, ot, wo : wo + 512],
                            in_=out_sb[:, ot, wo : wo + 512],
                        )

    nc.compile()
    return nc


_NC_CACHE = None


def _get_nc():
    global _NC_CACHE
    if _NC_CACHE is None:
        _NC_CACHE = build_kernel()
    return _NC_CACHE


def make_in_maps(x, w_qkv, w_out, b_out):
    bf16 = ml_dtypes.bfloat16
    wq = np.array(w_qkv, dtype=np.float32, copy=True)
    wq[:HID] *= SCALE  # fold attention scale into the q projection
    wqkvT = np.ascontiguousarray(wq.T)  # [256, 1536]
    # duplicated q/k head blocks: per head h, cols [wq_h | wq_h] (64+64)
    qd = wqkvT[:, 0:HID].reshape(DIM, HEADS, 1, DH)
    qdup = np.broadcast_to(qd, (DIM, HEADS, 2, DH)).reshape(DIM, 2 * HID)
    kd = wqkvT[:, HID : 2 * HID].reshape(DIM, HEADS, 1, DH)
    kdup = np.broadcast_to(kd, (DIM, HEADS, 2, DH)).reshape(DIM, 2 * HID)
    wqkv = np.concatenate([qdup, kdup, wqkvT[:, 2 * HID :]], axis=1)
    wqkv = np.ascontiguousarray(wqkv).astype(bf16)  # [256, 2560]
    woutT = np.ascontiguousarray(
        w_out.T.reshape(4, 128, DIM).transpose(1, 0, 2)
    ).astype(bf16)  # [128, 4, 256]
    bias = np.ascontiguousarray(
        b_out.astype(np.float32).reshape(DIM // 128, 128).T
    )  # [128, 2]
    in_maps = []
    for i in range(N_CORES):
        in_maps.append(
            {
                "x": x[i].astype(bf16),
                "wqkv": wqkv,
                "woutT": woutT,
                "bias": bias,
            }
        )
    return in_maps


def kernel(x, w_qkv, w_out, b_out, _trace=False):
    nc = _get_nc()
    in_maps = make_in_maps(x, w_qkv, w_out, b_out)
    res = run_bass_kernel_spmd(
        nc,
        in_maps,
        core_ids=list(range(N_CORES)),
        trace=_trace,
        trace_cores=list(range(N_CORES)) if _trace else None,
    )
    out = np.stack([res.results[i]["out"] for i in range(N_CORES)], axis=0)
    if _trace:
        kernel.last_exec_time_ns = res.exec_time_ns
        kernel.last_results = res
    return out


# revision 5
# speedup vs baseline: 1.8451x; 1.3757x over previous
"""Trainium2 Bass kernel for nn_Attention_15539191677265.

Single-head-dim attention block:
    qkv = w_qkv @ x ; per-head scaled dot-product attention over w=2048;
    out = w_out @ attn_out + b_out

Sharding: pure data-parallel over batch (b=8 -> 8 NeuronCores, one batch
element per core). Weights are replicated. No collectives.

Per-core algorithm (transposed-softmax, bf16 matmuls, ACT-bound pipeline):
  1. q,k projections emit PARTITION-DUPLICATED copies (d-dims at
     partitions 0-63 and 64-127) via duplicated stationary weights, so the
     sim matmuls can run as two concurrent PE quadrant streams.
  2. vT = x.T @ wvT with a ones column appended (row 64 of the AV psum
     accumulates the softmax normalizer for free).
  3. per (head, i-half 1024, j-tile 128): sim^T strip via two K=64
     matmuls on PE quadrants (0,0) and (64,64) -> [128 j, 1024 i] psum;
     exp on ScalarE as ONE [128,1024] activation (2-bank psum span);
     AV accumulates [65, 1024] over the 16 j-tiles.
  4. normalize via reciprocal of the ones-row, broadcast multiply (DVE /
     GpSimd, off the critical path), bounce odd heads via DMA.
  5. proj: out = woutT.T @ attn_out (K=128 head-pair chunks) + bias.

No max subtraction before exp: scores are ~N(0,1) so exp cannot
overflow in bf16/fp32.
"""

import sys

if "/opt/trn_rl_repo" not in sys.path:
    sys.path.insert(0, "/opt/trn_rl_repo")

import numpy as np
import ml_dtypes

import concourse.bass as bass
import concourse.mybir as mybir
import concourse.tile as tile
from concourse import bacc
from concourse.bass_utils import run_bass_kernel_spmd

BF16 = mybir.dt.bfloat16
F32 = mybir.dt.float32
EXP = mybir.ActivationFunctionType.Exp

B, DIM, W = 8, 256, 2048
HEADS, DH = 8, 64
HID = HEADS * DH  # 512
SCALE = DH ** (-0.5)
N_CORES = 8

NJT = W // 128  # 16 j-tiles per head
NCT = DIM // 128  # 2 contraction chunks over channels
IH = 1024  # i-half width
NIH = W // IH  # 2 i-halves


def build_kernel():
    nc = bacc.Bacc(None, target_bir_lowering=False)

    x_d = nc.dram_tensor("x", [DIM, W], BF16, kind="ExternalInput")
    # host layout: [256, 2560] = [qdup 1024 | kdup 1024 | wv 512]
    wqkv_d = nc.dram_tensor("wqkv", [DIM, 2560], BF16, kind="ExternalInput")
    woutT_d = nc.dram_tensor("woutT", [128, 4, DIM], BF16, kind="ExternalInput")
    bias_d = nc.dram_tensor("bias", [128, DIM // 128], F32, kind="ExternalInput")
    out_d = nc.dram_tensor("out", [DIM, W], F32, kind="ExternalOutput")

    with tile.TileContext(nc) as tc:
        with tc.tile_pool(name="pers", bufs=1) as pers:
            x_sb = pers.tile([128, NCT, W], BF16)
            wq_sb = pers.tile([128, NCT, 2560], BF16)
            wo_sb = pers.tile([128, 4, DIM], BF16)
            bias_sb = pers.tile([128, DIM // 128], F32)
            # duplicated-partition layouts: [128 = d 0..63 twice, head, w]
            q_sb = pers.tile([128, HEADS, W], BF16)
            k_sb = pers.tile([128, HEADS, W], BF16)
            vt_sb = pers.tile([128, NJT, HEADS, 128], BF16)
            attout_sb = [
                pers.tile([128, W], BF16, name=f"attout_{kc}", tag=f"attout{kc}")
                for kc in range(4)
            ]
            out_sb = pers.tile([128, NCT, W], F32)

            xr = x_d[:].rearrange("(ct p) w -> p ct w", p=128)
            for ct in range(NCT):
                for wh in range(4):
                    nc.sync.dma_start(
                        out=x_sb[:, ct, wh * 512 : (wh + 1) * 512],
                        in_=xr[:, ct, wh * 512 : (wh + 1) * 512],
                    )
            wqr = wqkv_d[:].rearrange("(ct p) o -> p ct o", p=128)
            for ct in range(NCT):
                for sec in range(5):
                    nc.sync.dma_start(
                        out=wq_sb[:, ct, sec * 512 : (sec + 1) * 512],
                        in_=wqr[:, ct, sec * 512 : (sec + 1) * 512],
                    )
            nc.sync.dma_start(out=wo_sb[:], in_=woutT_d[:])
            nc.sync.dma_start(out=bias_sb[:], in_=bias_d[:])

            # vT cols 64..127: ones column then zero padding
            nc.vector.memset(vt_sb[:, :, :, DH:128], 0.0)
            nc.vector.memset(vt_sb[:, :, :, DH : DH + 1], 1.0)
            # warm the ACT exp table while qkv matmuls run
            warm = pers.tile([1, 1], F32)
            nc.vector.memset(warm[:], 0.0)
            nc.scalar.activation(out=warm[:], in_=warm[:], func=EXP)

            # ---- phase 1: q, k projections (partition-duplicated) ----
            with tc.tile_pool(name="qkv_ps", bufs=4, space="PSUM") as qkv_ps:
                for h in range(HEADS):
                    for dst, base in ((q_sb, 0), (k_sb, 1024)):
                        for ph in range(4):
                            po = ph * 512
                            ps = qkv_ps.tile([128, 512], F32, name=f"qk_{h}_{base}_{ph}", tag="qk")
                            for ct in range(NCT):
                                nc.tensor.matmul(
                                    ps[:],
                                    lhsT=wq_sb[:, ct, base + h * 128 : base + (h + 1) * 128],
                                    rhs=x_sb[:, ct, po : po + 512],
                                    start=(ct == 0),
                                    stop=(ct == NCT - 1),
                                )
                            if ph % 2 == 0:
                                nc.vector.tensor_copy(
                                    out=dst[:, h, po : po + 512], in_=ps[:]
                                )
                            else:
                                nc.scalar.copy(
                                    out=dst[:, h, po : po + 512], in_=ps[:]
                                )

                # ---- phase 2: vT[j, hd] = x.T @ wvT (+ones col kept) ----
                for jt in range(NJT):
                    ps = qkv_ps.tile([128, HID], F32, name=f"vt_{jt}", tag="vt")
                    for ct in range(NCT):
                        nc.tensor.matmul(
                            ps[:],
                            lhsT=x_sb[:, ct, jt * 128 : (jt + 1) * 128],
                            rhs=wq_sb[:, ct, 2048:2560],
                            start=(ct == 0),
                            stop=(ct == NCT - 1),
                        )
                    if jt % 2 == 0:
                        nc.vector.tensor_copy(
                            out=vt_sb[:, jt, :, 0:DH],
                            in_=ps[:].rearrange("p (h d) -> p h d", h=HEADS),
                        )
                    else:
                        nc.scalar.copy(
                            out=vt_sb[:, jt, :, 0:DH],
                            in_=ps[:].rearrange("p (h d) -> p h d", h=HEADS),
                        )

            # ---- phase 3: attention ----
            with (
                tc.tile_pool(name="strip_ps", bufs=2, space="PSUM") as strip_ps,
                tc.tile_pool(name="av_ps", bufs=2, space="PSUM") as av_ps,
                tc.tile_pool(name="exp_sb", bufs=3) as exp_pool,
                tc.tile_pool(name="norm_sb", bufs=2) as norm_pool,
            ):
                for h in range(HEADS):
                    for ih in range(NIH):
                        io = ih * IH
                        av = av_ps.tile([128, IH], F32, name=f"av_{h}_{ih}", tag="av")
                        for jt in range(NJT):
                            strip = strip_ps.tile(
                                [128, IH], F32, name=f"st_{h}_{ih}_{jt}", tag="st"
                            )
                            for c in range(IH // 512):
                                co = c * 512
                                # full-width K=64 matmuls on alternating PE
                                # row-halves: adjacent ops use disjoint rows
                                # and different psum banks, so they stream
                                # concurrently (2x column rate)
                                rg = 64 * (c % 2)
                                nc.tensor.matmul(
                                    strip[:, co : co + 512],
                                    lhsT=k_sb[rg : rg + 64, h, jt * 128 : (jt + 1) * 128],
                                    rhs=q_sb[rg : rg + 64, h, io + co : io + co + 512],
                                    start=True,
                                    stop=True,
                                    tile_position=(rg, 0),
                                )
                            es = exp_pool.tile([128, IH], BF16, name=f"es_{jt}", tag="es")
                            nc.scalar.activation(out=es[:], in_=strip[:], func=EXP)
                            for c in range(IH // 512):
                                co = c * 512
                                nc.tensor.matmul(
                                    av[0 : DH + 1, co : co + 512],
                                    lhsT=vt_sb[:, jt, h, 0 : DH + 1],
                                    rhs=es[:, co : co + 512],
                                    start=(jt == 0),
                                    stop=(jt == NJT - 1),
                                )
                        # normalize off the critical path
                        avc = norm_pool.tile([DH + 1, IH], F32, tag="avc")
                        nc.vector.tensor_copy(out=avc[:], in_=av[0 : DH + 1, :])
                        rec0 = norm_pool.tile([1, IH], F32, tag="rec0")
                        bcn = norm_pool.tile([DH, IH], F32, tag="bcn")
                        bc = norm_pool.tile([DH, IH], F32, tag="bc")
                        nc.sync.dma_start(out=rec0[:], in_=avc[DH : DH + 1, :])
                        nc.gpsimd.partition_broadcast(bcn[:], rec0[0:1, :], channels=DH)
                        nc.vector.reciprocal_approx_fast(out=bc[:], in_=bcn[:])
                        if h % 2 == 0:
                            nc.vector.tensor_mul(
                                out=attout_sb[h // 2][0:DH, io : io + IH],
                                in0=avc[0:DH, :],
                                in1=bc[:],
                            )
                        else:
                            # odd heads land on partitions 64..127: DVE cannot
                            # shift partitions, so bounce + DMA
                            atmp = norm_pool.tile([DH, IH], BF16, tag="atmp")
                            nc.vector.tensor_mul(out=atmp[:], in0=avc[0:DH, :], in1=bc[:])
                            nc.sync.dma_start(
                                out=attout_sb[h // 2][DH:128, io : io + IH],
                                in_=atmp[:],
                            )

            # ---- phase 4: output projection + bias ----
            outr = out_d[:].rearrange("(ct p) w -> p ct w", p=128)
            with tc.tile_pool(name="proj_ps", bufs=8, space="PSUM") as proj_ps:
                for ot in range(NCT):
                    for wh in range(4):
                        wo = wh * 512
                        ps = proj_ps.tile([128, 512], F32, name=f"pj_{ot}_{wh}", tag="pj")
                        for kc in range(4):
                            nc.tensor.matmul(
                                ps[:],
                                lhsT=wo_sb[:, kc, ot * 128 : (ot + 1) * 128],
                                rhs=attout_sb[kc][:, wo : wo + 512],
                                start=(kc == 0),
                                stop=(kc == 3),
                            )
                        nc.vector.tensor_scalar_add(
                            out=out_sb[:, ot, wo : wo + 512],
                            in0=ps[:],
                            scalar1=bias_sb[:, ot : ot + 1],
                        )
                        nc.sync.dma_start(
                            out=outr[:, ot, wo : wo + 512],
                            in_=out_sb[:, ot, wo : wo + 512],
                        )

    nc.compile()
    return nc


_NC_CACHE = None


def _get_nc():
    global _NC_CACHE
    if _NC_CACHE is None:
        _NC_CACHE = build_kernel()
    return _NC_CACHE


def make_in_maps(x, w_qkv, w_out, b_out):
    bf16 = ml_dtypes.bfloat16
    wq = np.array(w_qkv, dtype=np.float32, copy=True)
    wq[:HID] *= SCALE  # fold attention scale into the q projection
    wqkvT = np.ascontiguousarray(wq.T)  # [256, 1536]
    # duplicated q/k head blocks: per head h, cols [wq_h | wq_h] (64+64)
    qd = wqkvT[:, 0:HID].reshape(DIM, HEADS, 1, DH)
    qdup = np.broadcast_to(qd, (DIM, HEADS, 2, DH)).reshape(DIM, 2 * HID)
    kd = wqkvT[:, HID : 2 * HID].reshape(DIM, HEADS, 1, DH)
    kdup = np.broadcast_to(kd, (DIM, HEADS, 2, DH)).reshape(DIM, 2 * HID)
    wqkv = np.concatenate([qdup, kdup, wqkvT[:, 2 * HID :]], axis=1)
    wqkv = np.ascontiguousarray(wqkv).astype(bf16)  # [256, 2560]
    woutT = np.ascontiguousarray(
        w_out.T.reshape(4, 128, DIM).transpose(1, 0, 2)
    ).astype(bf16)  # [128, 4, 256]
    bias = np.ascontiguousarray(
        b_out.astype(np.float32).reshape(DIM // 128, 128).T
    )  # [128, 2]
    in_maps = []
    for i in range(N_CORES):
        in_maps.append(
            {
                "x": x[i].astype(bf16),
                "wqkv": wqkv,
                "woutT": woutT,
                "bias": bias,
            }
        )
    return in_maps


def kernel(x, w_qkv, w_out, b_out, _trace=False):
    nc = _get_nc()
    in_maps = make_in_maps(x, w_qkv, w_out, b_out)
    res = run_bass_kernel_spmd(
        nc,
        in_maps,
        core_ids=list(range(N_CORES)),
        trace=_trace,
        trace_cores=list(range(N_CORES)) if _trace else None,
    )
    out = np.stack([res.results[i]["out"] for i in range(N_CORES)], axis=0)
    if _trace:
        kernel.last_exec_time_ns = res.exec_time_ns
        kernel.last_results = res
    return out
